# revision 8
# baseline (speedup 1.0000x reference)
"""DiffLlama flash-attention-2 block for Trainium2, 8-core tensor-parallel.

Reference computation (B=1, S=2048, HID=2048, H=32 q-heads, HKV=8 kv-heads,
D=64): q/k/v projections, RoPE, GQA, differential attention (16 effective
head-pairs, each pair = softmax1 @ [vL|vH] - lam * softmax2 @ [vL|vH]),
RMS-norm over the 2D=128 feature dim, output projection.

Sharding: effective head pairs {2c, 2c+1} on core c (tensor parallel).
Each core gets the full hidden states (pre-transposed on host), its 4 q-heads,
2 kv-heads, and a 256-column slice of Wo; partial outputs are summed on host.

All matmuls run in float32r (full PE rate at moving-dim >= 256, ~16-bit
effective mantissa). Scores are computed transposed (ST[sk,sq] = k q^T) so
softmax denominators and PV come out of ones-vector / v^T matmuls without any
on-chip transposes of the probability matrix.
"""

import math
from contextlib import ExitStack

import numpy as np

import concourse.bass as bass
import concourse.mybir as mybir
import concourse.tile as tile
from concourse._compat import with_exitstack
from concourse.bass_utils import run_bass_kernel_spmd
from concourse.masks import make_identity

# problem geometry
HID = 2048
H = 32
HKV = 8
D = 64
EPS = 1e-6
LAYER_IDX = 8
LAMBDA_INIT = float(0.8 - 0.6 * math.exp(-0.3 * LAYER_IDX))
NCORES = 8

F32 = mybir.dt.float32
F32R = mybir.dt.float32r
AF = mybir.ActivationFunctionType
ALU = mybir.AluOpType

_NC_CACHE = {}


# ---------------------------------------------------------------------------
# BIR post-pass: some instruction structs (S3_LW for matmul/ldweights, CTRL_NO
# for drain, PSEUDO_DMA for DMA triggers) only hold ONE sync-wait in this
# walrus build. Hoist excess waits onto single-wait NOPs on the same engine.
# ---------------------------------------------------------------------------
SINGLE_WAIT_OPS = {
    "InstMatmult",
    "InstLdweights",
    "InstDrain",
    "InstDMACopy",
    "InstDmaTransposeAnt",
}


def fix_single_wait_ops(nc):
    eng_map = {
        mybir.EngineType.PE: nc.tensor,
        mybir.EngineType.DVE: nc.vector,
        mybir.EngineType.Activation: nc.scalar,
        mybir.EngineType.Pool: nc.gpsimd,
        mybir.EngineType.SP: nc.sync,
    }
    n_fixed = 0
    for f in nc.m.functions:
        for bb in f.blocks:
            new_insts = []
            for inst in bb.instructions:
                si = getattr(inst, "sync_info", None)
                if (
                    type(inst).__name__ != "InstNoOp"
                    and si is not None
                    and si.on_wait
                    and len(si.on_wait) > 1
                ):
                    eng = eng_map[inst.engine]
                    for w in list(si.on_wait[:-1]):
                        nop = eng.nop(nofuse=True).ins
                        for fb in nc.m.functions:
                            for bb2 in fb.blocks:
                                if bb2.instructions and bb2.instructions[-1] is nop:
                                    bb2.instructions.pop()
                        nop.sync_info = mybir.SyncInfo(on_wait=[w], on_update=[])
                        new_insts.append(nop)
                    inst.sync_info = mybir.SyncInfo(
                        on_wait=list(si.on_wait[-1:]), on_update=list(si.on_update)
                    )
                    n_fixed += 1
                new_insts.append(inst)
            bb.instructions[:] = new_insts
    return n_fixed


# ---------------------------------------------------------------------------
# device kernel
# ---------------------------------------------------------------------------
@with_exitstack
def _diffllama_kernel(ctx: ExitStack, tc: tile.TileContext, S: int, io: dict):
    nc = tc.nc
    SB = 512            # sq block
    NB = S // SB        # sq blocks
    NT = S // 128       # sk tiles
    KT = HID // 128     # contraction tiles for projections
    C1 = 1.0 - LAMBDA_INIT

    hT, wqT, wkT, wvT, woT = io["hT"], io["wqT"], io["wkT"], io["wvT"], io["woT"]
    cos2, sinS, neglam, y = io["cos2"], io["sinS"], io["neglam"], io["y"]
    onesc = io["onesc"]

    consts = ctx.enter_context(tc.tile_pool(name="consts", bufs=1))
    weights = ctx.enter_context(tc.tile_pool(name="weights", bufs=1))
    persist = ctx.enter_context(tc.tile_pool(name="persist", bufs=1))
    hpool = ctx.enter_context(tc.tile_pool(name="hpool", bufs=6))
    work = ctx.enter_context(tc.tile_pool(name="work", bufs=2))
    bwork = ctx.enter_context(tc.tile_pool(name="bwork", bufs=3))
    dscratch = ctx.enter_context(tc.tile_pool(name="dscratch", bufs=4, space="DRAM"))

    def bcast(vec, tag, name):
        """[1, SB] f32 SBUF -> [128, SB] f32 SBUF via DRAM roundtrip."""
        dr = dscratch.tile([1, SB], F32, tag="dr", name=f"dr_{name}")
        nc.sync.dma_start(out=dr, in_=vec)
        out = bwork.tile([128, SB], F32, tag=tag, name=name)
        nc.sync.dma_start(
            out=out,
            in_=bass.AP(tensor=dr.tensor, offset=dr.offset, ap=[[0, 128], [1, SB]]),
        )
        return out
    etpool = ctx.enter_context(tc.tile_pool(name="etpool", bufs=3))
    upool = ctx.enter_context(tc.tile_pool(name="upool", bufs=4))
    vecs = ctx.enter_context(tc.tile_pool(name="vecs", bufs=2))
    ps_mm = ctx.enter_context(tc.tile_pool(name="ps_mm", bufs=2, space="PSUM"))
    ps_st = ctx.enter_context(tc.tile_pool(name="ps_st", bufs=2, space="PSUM"))
    ps_ut = ctx.enter_context(tc.tile_pool(name="ps_ut", bufs=2, space="PSUM"))
    ps_sm = ctx.enter_context(tc.tile_pool(name="ps_sm", bufs=2, space="PSUM"))

    # --- constants / weights ---
    wq_sb = weights.tile([128, KT, 2 * D * 2], F32R)   # [128, 16, 256]
    wk_sb = weights.tile([128, KT, 2 * D], F32R)
    wv_sb = weights.tile([128, KT, 2 * D], F32R)
    wo_sb = weights.tile([128, 2, HID], F32R)
    nc.sync.dma_start(out=wq_sb, in_=wqT.rearrange("(kt p) m -> p kt m", p=128))
    nc.sync.dma_start(out=wk_sb, in_=wkT.rearrange("(kt p) m -> p kt m", p=128))
    nc.sync.dma_start(out=wv_sb, in_=wvT.rearrange("(kt p) m -> p kt m", p=128))
    nc.sync.dma_start(out=wo_sb, in_=woT.rearrange("(kt p) m -> p kt m", p=128))
    neglam_sb = consts.tile([128, 1], F32)
    nc.sync.dma_start(
        out=neglam_sb,
        in_=bass.AP(tensor=neglam.tensor, offset=0, ap=[[0, 128], [1, 1]]),
    )
    ones_sb = consts.tile([128, 1], F32R)
    nc.sync.dma_start(out=ones_sb, in_=onesc)
    eps_sb = consts.tile([1, 1], F32)
    nc.vector.memset(eps_sb, EPS)
    ident = consts.tile([128, 128], F32)
    make_identity(nc, ident)

    kLdup = persist.tile([128, S], F32R)   # k side-1 roped, duplicated rows
    kHdup = persist.tile([128, S], F32R)
    vcat = persist.tile([128, NT, 128], F32R)
    qrope = [None, None]

    h3 = hT.rearrange("(kt p) s -> kt p s", p=128)

    def load_h(k, b):
        ht = hpool.tile([128, SB], F32R, tag="ht", name=f"ht_{k}_{b}")
        nc.sync.dma_start(out=ht, in_=h3[k, :, b * SB:(b + 1) * SB])
        return ht

    def rope(raw, cos_blk, sin_blk, rtag):
        """raw [128, SB] f32 -> roped [128, SB] f32r (rows = 2 heads of 64)."""
        swp = work.tile([128, SB], F32, tag="swp", bufs=1)
        nc.sync.dma_start(out=swp[0:32, :], in_=raw[32:64, :])
        nc.sync.dma_start(out=swp[32:64, :], in_=raw[0:32, :])
        nc.sync.dma_start(out=swp[64:96, :], in_=raw[96:128, :])
        nc.sync.dma_start(out=swp[96:128, :], in_=raw[64:96, :])
        m1 = work.tile([128, SB], F32, tag="m1", bufs=1)
        nc.vector.tensor_mul(m1, raw, cos_blk)
        out = work.tile([128, SB], F32R, tag=rtag)
        m2 = work.tile([128, SB], F32, tag="m2", bufs=1)
        nc.vector.tensor_mul(m2, swp, sin_blk)
        nc.vector.tensor_add(out, m1, m2)
        return out

    for b in range(NB):
        bs = slice(b * SB, (b + 1) * SB)
        ntile = (b + 1) * (SB // 128)

        # ---- projections: pass A (q), pass B (k, v) ----
        raws = {}
        for pass_groups in (("qA", "qB"), ("kr", "vr")):
            pss = {}
            for g in pass_groups:
                pss[g] = ps_mm.tile([128, SB], F32, tag="mm", name=f"ps_{g}_{b}")
            for k in range(KT):
                ht = load_h(k, b)
                for g in pass_groups:
                    if g == "qA":
                        lhsT = wq_sb[:, k, 0:128]
                    elif g == "qB":
                        lhsT = wq_sb[:, k, 128:256]
                    elif g == "kr":
                        lhsT = wk_sb[:, k, :]
                    else:
                        lhsT = wv_sb[:, k, :]
                    nc.tensor.matmul(
                        pss[g], lhsT, ht, start=(k == 0), stop=(k == KT - 1)
                    )
            for g in pass_groups:
                raw = work.tile([128, SB], F32, tag=f"raw_{g}", bufs=1)
                nc.scalar.copy(raw, pss[g])
                raws[g] = raw

        # ---- RoPE ----
        cos_blk = work.tile([128, SB], F32, tag="cosb", name=f"cosb_{b}")
        sin_blk = work.tile([128, SB], F32, tag="sinb", name=f"sinb_{b}")
        nc.sync.dma_start(out=cos_blk, in_=cos2[:, bs])
        nc.sync.dma_start(out=sin_blk, in_=sinS[:, bs])
        qrope[0] = rope(raws["qA"], cos_blk, sin_blk, "ropedA")
        qrope[1] = rope(raws["qB"], cos_blk, sin_blk, "ropedB")
        krope = rope(raws["kr"], cos_blk, sin_blk, "ropedK")
        nc.sync.dma_start(out=kLdup[0:64, bs], in_=krope[0:64, :])
        nc.sync.dma_start(out=kLdup[64:128, bs], in_=krope[0:64, :])
        nc.sync.dma_start(out=kHdup[0:64, bs], in_=krope[64:128, :])
        nc.sync.dma_start(out=kHdup[64:128, bs], in_=krope[64:128, :])

        # ---- V transpose -> vcat [s, feat] ----
        for t4 in range(SB // 128):
            t = b * (SB // 128) + t4
            tps = ps_mm.tile([128, 128], F32, tag="mm", name=f"tps_{t}")
            nc.tensor.transpose(tps, raws["vr"][:, t4 * 128:(t4 + 1) * 128], ident)
            nc.scalar.copy(vcat[:, t, :], tps)

        # ---- attention: sides sequential, effective heads paired ----
        u_sb = [[None, None], [None, None]]   # [side][e]
        r_sb = [[None, None], [None, None]]
        for x in (0, 1):
            kdup = (kLdup, kHdup)[x]
            q = qrope[x]
            ut = [
                ps_ut.tile([128, SB], F32, tag="ut", name=f"ut_{b}_{x}_{e}")
                for e in (0, 1)
            ]
            sm = [
                ps_sm.tile([1, SB], F32, tag="sm", name=f"sm_{b}_{x}_{e}")
                for e in (0, 1)
            ]
            for t in range(ntile):
                tsl = slice(t * 128, (t + 1) * 128)
                for e in (0, 1):
                    st = ps_st.tile([128, SB], F32, tag="st", name=f"st_{b}_{x}_{t}_{e}")
                    nc.tensor.matmul(
                        st,
                        kdup[64 * e:64 * (e + 1), tsl],
                        q[64 * e:64 * (e + 1), :],
                        start=True,
                        stop=True,
                    )
                    et = etpool.tile([128, SB], F32R, tag="et", name=f"et_{b}_{x}_{t}_{e}")
                    nc.scalar.activation(et, st, AF.Exp, scale=1.0 / math.sqrt(D))
                    if t >= b * (SB // 128):
                        # diagonal tile: zero out strictly-future keys
                        nc.gpsimd.affine_select(
                            out=et,
                            in_=et,
                            compare_op=ALU.is_ge,
                            fill=0.0,
                            base=b * SB - t * 128,
                            pattern=[[1, SB]],
                            channel_multiplier=-1,
                        )
                    nc.tensor.matmul(
                        ut[e], vcat[:, t, :], et,
                        start=(t == 0), stop=(t == ntile - 1),
                    )
                    nc.tensor.matmul(
                        sm[e], ones_sb, et,
                        start=(t == 0), stop=(t == ntile - 1),
                    )
            for e in (0, 1):
                u = upool.tile([128, SB], F32, tag="u", name=f"u_{b}_{x}_{e}")
                nc.scalar.copy(u, ut[e])
                u_sb[x][e] = u
                r = vecs.tile([1, SB], F32, tag="r", bufs=6, name=f"r_{b}_{x}_{e}")
                nc.vector.reciprocal(r, sm[e])
                r_sb[x][e] = r

        # ---- combine, RMS norm, output projection ----
        anrm = [None, None]
        for e in (0, 1):
            r1b = bcast(r_sb[0][e], "rb", f"r1b_{b}_{e}")
            a1 = work.tile([128, SB], F32, tag="cmb", bufs=3, name=f"a1_{b}_{e}")
            nc.vector.tensor_mul(a1, u_sb[0][e], r1b)
            r2b = bcast(r_sb[1][e], "rb", f"r2b_{b}_{e}")
            a2 = work.tile([128, SB], F32, tag="cmb", bufs=3, name=f"a2_{b}_{e}")
            nc.vector.tensor_mul(a2, u_sb[1][e], r2b)
            aT = work.tile([128, SB], F32, tag="aT")
            nc.vector.scalar_tensor_tensor(
                out=aT, in0=a2, scalar=neglam_sb[:, 0:1], in1=a1,
                op0=ALU.mult, op1=ALU.add,
            )
            sq = work.tile([128, SB], F32R, tag="sq", bufs=1, name=f"sq_{b}_{e}")
            nc.vector.tensor_mul(sq, aT, aT)
            msq = ps_sm.tile([1, SB], F32, tag="sm", name=f"msq_{b}_{e}")
            nc.tensor.matmul(msq, ones_sb, sq, start=True, stop=True)
            rstd = vecs.tile([1, SB], F32, tag="rstd", bufs=2)
            nc.scalar.activation(rstd, msq, AF.Sqrt, scale=1.0 / 128.0, bias=eps_sb[0:1, 0:1])
            rr = vecs.tile([1, SB], F32, tag="rr", bufs=2)
            nc.vector.reciprocal(rr, rstd)
            rrb = bcast(rr, "rb", f"rrb_{b}_{e}")
            an = work.tile([128, SB], F32R, tag=f"anrm{e}")
            nc.vector.scalar_tensor_tensor(
                out=an, in0=aT, scalar=C1, in1=rrb,
                op0=ALU.mult, op1=ALU.mult,
            )
            anrm[e] = an

        for sq_sub in range(SB // 128):
            for ob in range(HID // SB):
                yps = ps_mm.tile([128, SB], F32, tag="mm", name=f"yps_{b}_{sq_sub}_{ob}")
                for e in (0, 1):
                    nc.tensor.matmul(
                        yps,
                        anrm[e][:, sq_sub * 128:(sq_sub + 1) * 128],
                        wo_sb[:, e, ob * SB:(ob + 1) * SB],
                        start=(e == 0),
                        stop=(e == 1),
                    )
                ysb = work.tile([128, SB], F32, tag="ysb")
                nc.scalar.copy(ysb, yps)
                r0 = b * SB + sq_sub * 128
                nc.sync.dma_start(
                    out=y[r0:r0 + 128, ob * SB:(ob + 1) * SB], in_=ysb
                )


def _build_nc(S=2048):
    if S in _NC_CACHE:
        return _NC_CACHE[S]
    nc = bass.Bass(num_devices=NCORES)
    io = {
        "hT": nc.dram_tensor("hT", [HID, S], F32R, kind="ExternalInput").ap(),
        "wqT": nc.dram_tensor("wqT", [HID, 256], F32R, kind="ExternalInput").ap(),
        "wkT": nc.dram_tensor("wkT", [HID, 128], F32R, kind="ExternalInput").ap(),
        "wvT": nc.dram_tensor("wvT", [HID, 128], F32R, kind="ExternalInput").ap(),
        "woT": nc.dram_tensor("woT", [256, HID], F32R, kind="ExternalInput").ap(),
        "cos2": nc.dram_tensor("cos2", [128, S], F32, kind="ExternalInput").ap(),
        "sinS": nc.dram_tensor("sinS", [128, S], F32, kind="ExternalInput").ap(),
        "neglam": nc.dram_tensor("neglam", [1, 1], F32, kind="ExternalInput").ap(),
        "onesc": nc.dram_tensor("onesc", [128, 1], F32R, kind="ExternalInput").ap(),
        "y": nc.dram_tensor("y", [S, HID], F32, kind="ExternalOutput").ap(),
    }
    with tile.TileContext(nc) as tc:
        _diffllama_kernel(tc, S, io)
    fix_single_wait_ops(nc)
    _NC_CACHE[S] = nc
    return nc


# ---------------------------------------------------------------------------
# host-side sharding + driver
# ---------------------------------------------------------------------------
def make_in_maps(hidden_states, cos, sin, Wq, Wk, Wv, Wo,
                 lambda_q1, lambda_k1, lambda_q2, lambda_k2, S):
    h = np.asarray(hidden_states, np.float32).reshape(S, HID)
    hT = np.ascontiguousarray(h.T)
    cos = np.asarray(cos, np.float32)
    sin = np.asarray(sin, np.float32)
    cos2 = np.ascontiguousarray(np.tile(cos.T, (2, 1)))
    sinS64 = np.concatenate([-sin.T[:32], sin.T[32:]], axis=0)
    sinS = np.ascontiguousarray(np.tile(sinS64, (2, 1)))

    lam1 = np.exp(np.sum(np.asarray(lambda_q1, np.float64) * np.asarray(lambda_k1, np.float64)))
    lam2 = np.exp(np.sum(np.asarray(lambda_q2, np.float64) * np.asarray(lambda_k2, np.float64)))
    lam = np.float32(lam1 - lam2 + LAMBDA_INIT)
    neglam = np.full((1, 1), -lam, np.float32)

    Wq3 = np.asarray(Wq, np.float32).reshape(H, D, HID)
    Wk3 = np.asarray(Wk, np.float32).reshape(HKV, D, HID)
    Wv3 = np.asarray(Wv, np.float32).reshape(HKV, D, HID)
    Wo = np.asarray(Wo, np.float32)

    in_maps = []
    for c in range(NCORES):
        e0, e1 = 2 * c, 2 * c + 1
        kvL, kvH = c // 2, c // 2 + 4
        qheads = [e0, e1, e0 + 16, e1 + 16]
        wqT = np.ascontiguousarray(Wq3[qheads].reshape(4 * D, HID).T)
        wkT = np.ascontiguousarray(Wk3[[kvL, kvH]].reshape(2 * D, HID).T)
        wvT = np.ascontiguousarray(Wv3[[kvL, kvH]].reshape(2 * D, HID).T)
        cols = np.r_[e0 * 128:(e0 + 1) * 128, e1 * 128:(e1 + 1) * 128]
        woT = np.ascontiguousarray(Wo[:, cols].T)
        in_maps.append({
            "hT": hT, "wqT": wqT, "wkT": wkT, "wvT": wvT, "woT": woT,
            "cos2": cos2, "sinS": sinS, "neglam": neglam,
            "onesc": np.ones((128, 1), np.float32),
        })
    return in_maps


def kernel(hidden_states, cos, sin, Wq, Wk, Wv, Wo,
           lambda_q1, lambda_k1, lambda_q2, lambda_k2):
    B, S, _ = hidden_states.shape
    nc = _build_nc(S)
    in_maps = make_in_maps(hidden_states, cos, sin, Wq, Wk, Wv, Wo,
                           lambda_q1, lambda_k1, lambda_q2, lambda_k2, S)
    res = run_bass_kernel_spmd(nc, in_maps, core_ids=list(range(NCORES)))
    y = np.zeros((S, HID), np.float64)
    for c in range(NCORES):
        y += res.results[c]["y"].astype(np.float64)
    return y.astype(np.float32).reshape(B, S, HID)
